# revision 2
# baseline (speedup 1.0000x reference)
"""Causal GQA attention (nkv=1) with RoPE + logit softcap, sharded over 8 trn2 cores.

Sharding: core = 2*b + hh  (b = batch 0..3, hh = head-half 0..1).
Each core computes, for its batch b and its 4 query heads:
  q = rope(x @ Wq_h'.T)          (gain/(sqrt(hd)*softcap) folded into Wq on host)
  k = rope(x @ Wk.T), v = x @ Wv.T   (single kv head, shared across its 4 q heads)
  pT[k,q] = exp(softcap*tanh(qT.k) - softcap) * causal_mask   (max-free softmax:
            softcap bounds logits to +-30 so exp never overflows)
  outT_h = (v.T @ pT) / sum_k pT    accumulated in PSUM; denominator via ones-matmul
  partial_out[tok, :] = sum_h outT_h.T @ Wo[:, head cols].T
Host sums the two half-head partials per batch and stacks batches.

All matmuls run as float32r (1 cycle/row at N=512, ~1e-4 rel err on hw).
"""
import numpy as np

import concourse.bacc as bacc
import concourse.mybir as mybir
import concourse.tile as tile
from concourse.bass_utils import run_bass_kernel_spmd
from concourse.masks import make_identity

F32 = mybir.dt.float32
F32R = mybir.dt.float32r

B, T, D = 4, 2048, 1024
NH, NKV, HD = 8, 1, 128
SOFTCAP = 30.0
ROPE_BASE = 500000.0
NHL = 4            # heads per core
CH = 512           # q-chunk size
NCH = T // CH      # 4 chunks
NKT = D // 128     # 8 k-tiles over D
NTT = T // 128     # 16 token tiles


def _build_nc():
    nc = bacc.Bacc()

    xT = nc.dram_tensor("xT", [D, T], F32, kind="ExternalInput")
    wqT = nc.dram_tensor("wqT", [D, NHL * HD], F32, kind="ExternalInput")
    wkT = nc.dram_tensor("wkT", [D, HD], F32, kind="ExternalInput")
    wvT = nc.dram_tensor("wvT", [D, HD], F32, kind="ExternalInput")
    woT = nc.dram_tensor("woT", [NHL * HD, D], F32, kind="ExternalInput")
    cc = nc.dram_tensor("cc", [HD, T], F32, kind="ExternalInput")
    ss = nc.dram_tensor("ss", [HD, T], F32, kind="ExternalInput")
    masks = nc.dram_tensor("masks", [128, 4 * CH], F32, kind="ExternalInput")
    onesv = nc.dram_tensor("onesv", [128, 1], F32, kind="ExternalInput")
    out = nc.dram_tensor("out", [T, D], F32, kind="ExternalOutput")

    # DRAM views tiled by 128 along the leading dim.
    xT_t = xT.rearrange("(kt p) t -> p kt t", p=128)      # [128, 8, 2048]
    wqT_t = wqT.rearrange("(kt p) c -> p kt c", p=128)    # [128, 8, 512]
    wkT_t = wkT.rearrange("(kt p) c -> p kt c", p=128)    # [128, 8, 128]
    wvT_t = wvT.rearrange("(kt p) c -> p kt c", p=128)    # [128, 8, 128]
    woT_t = woT.rearrange("(h p) c -> p h c", p=128)      # [128, 4, 1024]

    with tile.TileContext(nc) as tc:
        with (
            tc.tile_pool(name="persist", bufs=1) as persist,
            tc.tile_pool(name="wpool", bufs=1) as wpool,
            tc.tile_pool(name="xc_pool", bufs=2) as xc_pool,
            tc.tile_pool(name="qt_pool", bufs=2) as qt_pool,
            tc.tile_pool(name="rope_pool", bufs=3) as rope_pool,
            tc.tile_pool(name="p_pool", bufs=3) as p_pool,
            tc.tile_pool(name="avn_pool", bufs=6) as avn_pool,
            tc.tile_pool(name="norm_pool", bufs=2) as norm_pool,
            tc.tile_pool(name="osb_pool", bufs=3) as osb_pool,
            tc.tile_pool(name="mm_pool", bufs=3, space="PSUM") as mm_pool,
            tc.tile_pool(name="acc_pool", bufs=2, space="PSUM") as acc_pool,
            tc.tile_pool(name="d_pool", bufs=1, space="PSUM") as d_pool,
            tc.tile_pool(name="pj_pool", bufs=2, space="PSUM") as pj_pool,
        ):
            # --- constants & weights ---
            wq_sb = wpool.tile([128, NKT, NHL * HD], F32R)
            wk_sb = wpool.tile([128, NKT, HD], F32R)
            wv_sb = wpool.tile([128, NKT, HD], F32R)
            wo_sb = wpool.tile([128, NHL, D], F32R)
            cc_sb = wpool.tile([HD, T], F32)
            ss_sb = wpool.tile([HD, T], F32)
            mask_sb = wpool.tile([128, 4 * CH], F32R)
            ones_sb = wpool.tile([128, 1], F32R)
            nc.sync.dma_start(wq_sb[:], wqT_t.bitcast(F32R))
            nc.sync.dma_start(wk_sb[:], wkT_t.bitcast(F32R))
            nc.sync.dma_start(wv_sb[:], wvT_t.bitcast(F32R))
            nc.sync.dma_start(wo_sb[:], woT_t.bitcast(F32R))
            nc.sync.dma_start(cc_sb[:], cc[:])
            nc.sync.dma_start(ss_sb[:], ss[:])
            nc.sync.dma_start(mask_sb[:], masks[:].bitcast(F32R))
            nc.sync.dma_start(ones_sb[:], onesv[:].bitcast(F32R))

            ident = persist.tile([128, 128], F32)
            make_identity(nc, ident[:])
            negcap = persist.tile([128, 1], F32)
            nc.gpsimd.memset(negcap[:], -SOFTCAP)

            # K^T [hd, T] and V [tok, hd] persist for the whole sequence.
            kT_sb = persist.tile([HD, T], F32R)
            v_sb = persist.tile([128, NTT, HD], F32R)

            def rope_to(dst_ap, src_ps, c):
                """dst = rope(src) for a [128, CH] chunk at token offset c*CH."""
                csl = slice(c * CH, (c + 1) * CH)
                swp = rope_pool.tile([128, CH], F32, tag="swp", name="swp")
                nc.vector.tensor_copy(swp[0:64, :], src_ps[64:128, :])
                nc.vector.tensor_copy(swp[64:128, :], src_ps[0:64, :])
                m1 = rope_pool.tile([128, CH], F32, tag="m1", name="m1")
                nc.vector.tensor_mul(m1[:], src_ps[:], cc_sb[:, csl])
                m2 = rope_pool.tile([128, CH], F32, tag="m2", name="m2")
                nc.vector.tensor_mul(m2[:], swp[:], ss_sb[:, csl])
                nc.vector.tensor_add(dst_ap, m1[:], m2[:])

            for c in range(NCH):
                csl = slice(c * CH, (c + 1) * CH)

                # ---- projections for this token chunk ----
                xc = xc_pool.tile([128, NKT, CH], F32R, tag="xc", name="xc")
                nc.sync.dma_start(xc[:], xT_t[:, :, csl].bitcast(F32R))

                # Q^T per head (rope'd); only this chunk's queries are needed.
                qt = qt_pool.tile([HD, NHL, CH], F32R, tag="qt", name="qt")
                for h in range(NHL):
                    q_ps = mm_pool.tile([HD, CH], F32, tag="mm", name="q_ps")
                    for kt in range(NKT):
                        nc.tensor.matmul(
                            q_ps[:], wq_sb[:, kt, h * HD:(h + 1) * HD],
                            xc[:, kt, :], start=(kt == 0), stop=(kt == NKT - 1))
                    rope_to(qt[:, h, :], q_ps, c)

                # K^T chunk (rope'd)
                k_ps = mm_pool.tile([HD, CH], F32, tag="mm", name="k_ps")
                for kt in range(NKT):
                    nc.tensor.matmul(k_ps[:], wk_sb[:, kt, :], xc[:, kt, :],
                                     start=(kt == 0), stop=(kt == NKT - 1))
                rope_to(kT_sb[:, csl], k_ps, c)

                # V chunk: compute V^T, then PE-transpose into [tok, hd] tiles.
                v_ps = mm_pool.tile([HD, CH], F32, tag="mm", name="v_ps")
                for kt in range(NKT):
                    nc.tensor.matmul(v_ps[:], wv_sb[:, kt, :], xc[:, kt, :],
                                     start=(kt == 0), stop=(kt == NKT - 1))
                vt_sb = rope_pool.tile([HD, CH], F32, tag="vt", name="vt_sb")
                nc.vector.tensor_copy(vt_sb[:], v_ps[:])
                for tt in range(CH // 128):
                    tp_ps = mm_pool.tile([128, 128], F32, tag="mm", name="tp_ps")
                    nc.tensor.transpose(
                        tp_ps[:], vt_sb[:, tt * 128:(tt + 1) * 128], ident[:])
                    nc.vector.tensor_copy(v_sb[:, c * 4 + tt, :], tp_ps[:])

                # ---- attention for this q-chunk ----
                nkb = 4 * (c + 1)
                avn_tiles = []
                for h in range(NHL):
                    av_ps = acc_pool.tile([HD, CH], F32, tag="av", name="av_ps")
                    d_ps = d_pool.tile([1, CH], F32, tag="d", name="d_ps")
                    for kb in range(nkb):
                        s_ps = mm_pool.tile([128, CH], F32, tag="mm", name="s_ps")
                        nc.tensor.matmul(
                            s_ps[:], kT_sb[:, kb * 128:(kb + 1) * 128],
                            qt[:, h, :], start=True, stop=True)
                        t_sb = p_pool.tile([128, CH], F32, tag="t", name="t_sb")
                        nc.scalar.activation(
                            t_sb[:], s_ps[:], mybir.ActivationFunctionType.Tanh)
                        p_sb = p_pool.tile([128, CH], F32R, tag="p", name="p_sb")
                        nc.scalar.activation(
                            p_sb[:], t_sb[:], mybir.ActivationFunctionType.Exp,
                            scale=SOFTCAP, bias=negcap[:])
                        if kb >= 4 * c:
                            r = kb - 4 * c
                            pm = p_pool.tile([128, CH], F32R, tag="pm", name="pm")
                            nc.vector.tensor_mul(
                                pm[:], p_sb[:], mask_sb[:, r * CH:(r + 1) * CH])
                            p_fin = pm
                        else:
                            p_fin = p_sb
                        nc.tensor.matmul(av_ps[:], v_sb[:, kb, :], p_fin[:],
                                         start=(kb == 0), stop=(kb == nkb - 1))
                        nc.tensor.matmul(d_ps[:], ones_sb[:], p_fin[:],
                                         start=(kb == 0), stop=(kb == nkb - 1))
                    dinv = norm_pool.tile([128, CH], F32, tag="dinv", name="dinv")
                    nc.vector.reciprocal(dinv[0:1, :], d_ps[:])
                    dbc = norm_pool.tile([128, CH], F32, tag="dbc", name="dbc")
                    nc.gpsimd.partition_broadcast(dbc[:], dinv[:])
                    avn = avn_pool.tile([HD, CH], F32R, tag="avn", name="avn")
                    nc.vector.tensor_mul(avn[:], av_ps[:], dbc[:])
                    avn_tiles.append(avn)

                # ---- Wo partials for this chunk: out[tok, :] = sum_h ----
                for tt in range(CH // 128):
                    for dc in range(2):
                        o_ps = pj_pool.tile([128, CH], F32, tag="pj", name="o_ps")
                        for h in range(NHL):
                            nc.tensor.matmul(
                                o_ps[:],
                                avn_tiles[h][:, tt * 128:(tt + 1) * 128],
                                wo_sb[:, h, dc * CH:(dc + 1) * CH],
                                start=(h == 0), stop=(h == NHL - 1))
                        o_sb = osb_pool.tile([128, CH], F32, tag="osb", name="o_sb")
                        nc.vector.tensor_copy(o_sb[:], o_ps[:])
                        nc.sync.dma_start(
                            out[c * CH + tt * 128: c * CH + (tt + 1) * 128,
                                dc * CH:(dc + 1) * CH], o_sb[:])

    nc.compile()
    return nc


_CACHED_NC = None


def _get_nc():
    global _CACHED_NC
    if _CACHED_NC is None:
        _CACHED_NC = _build_nc()
    return _CACHED_NC


def _host_inputs(x, Wq, Wk, Wv, Wo, qk_gain, cos, sin):
    """Build the 8 per-core input maps (all fp32, C-contiguous)."""
    x = np.asarray(x, np.float32)
    Wq = np.asarray(Wq, np.float32)
    Wk = np.asarray(Wk, np.float32)
    Wv = np.asarray(Wv, np.float32)
    Wo = np.asarray(Wo, np.float32)
    qk_gain = np.asarray(qk_gain, np.float32)
    cos = np.asarray(cos, np.float32)
    sin = np.asarray(sin, np.float32)

    scale = 1.0 / (np.sqrt(HD) * SOFTCAP)
    # Fold per-head gain and softcap scale into Wq rows.
    Wq_s = Wq * (qk_gain[:, None].repeat(HD, 1).reshape(NH * HD, 1) * scale)

    wkT = np.ascontiguousarray(Wk.T)
    wvT = np.ascontiguousarray(Wv.T)
    cosT = cos.T  # [64, T]
    sinT = sin.T
    cc = np.ascontiguousarray(np.concatenate([cosT, cosT], 0))          # [128, T]
    ss = np.ascontiguousarray(np.concatenate([-sinT, sinT], 0))         # [128, T]

    # Diagonal-band causal masks: for residue r, mask[kk, qq] = qq >= 128*r + kk
    qq = np.arange(CH)
    kk = np.arange(128)
    masks = np.empty((128, 4 * CH), np.float32)
    for r in range(4):
        masks[:, r * CH:(r + 1) * CH] = (qq[None, :] >= (128 * r + kk)[:, None])
    onesv = np.ones((128, 1), np.float32)

    xTs = [np.ascontiguousarray(x[b].T) for b in range(B)]
    in_maps = []
    for core in range(8):
        b, hh = divmod(core, 2)
        h0 = hh * NHL
        wqT = np.ascontiguousarray(Wq_s[h0 * HD:(h0 + NHL) * HD, :].T)
        woT = np.ascontiguousarray(Wo[:, h0 * HD:(h0 + NHL) * HD].T)
        in_maps.append({
            "xT": xTs[b], "wqT": wqT, "wkT": wkT, "wvT": wvT, "woT": woT,
            "cc": cc, "ss": ss, "masks": masks, "onesv": onesv,
        })
    return in_maps


def kernel(x, Wq, Wk, Wv, Wo, qk_gain, cos, sin, _trace=False):
    in_maps = _host_inputs(x, Wq, Wk, Wv, Wo, qk_gain, cos, sin)
    nc = _get_nc()
    res = run_bass_kernel_spmd(nc, in_maps, core_ids=list(range(8)),
                               trace=_trace)
    out = np.empty((B, T, D), np.float32)
    for b in range(B):
        out[b] = res.results[2 * b]["out"] + res.results[2 * b + 1]["out"]
    if _trace:
        kernel.last_exec_time_ns = res.exec_time_ns
        kernel.last_results = res
    return out


# revision 6
# speedup vs baseline: 1.1736x; 1.1736x over previous
"""Causal GQA attention (nkv=1) with RoPE + logit softcap, sharded over 8 trn2 cores.

Sharding: core = 2*b + hh  (b = batch 0..3, hh = head-half 0..1).
Each core computes, for its batch b and its 4 query heads:
  q = rope(x @ Wq_h'.T)          (gain/(sqrt(hd)*softcap) folded into Wq on host)
  k = rope(x @ Wk.T), v = x @ Wv.T   (single kv head, shared across its 4 q heads)
  pT[k,q] = exp(softcap*tanh(qT.k) - softcap) * causal_mask   (max-free softmax:
            softcap bounds logits to +-30 so exp never overflows)
  outT_h = (v.T @ pT) / sum_k pT    accumulated in PSUM; denominator via ones-matmul
  partial_out[tok, :] = sum_h outT_h.T @ Wo[:, head cols].T
Host sums the two half-head partials per batch and stacks batches.

v2: all matmuls in bf16 (1 cyc/row, keeps the PE HAM-warm; fp32r lowers to
fp32_mode=HIGH which runs ~2x slower and HAM-oscillates). Scores accumulate in
fp32 PSUM; softmax numerics (tanh/exp on ACT, fp32 in) stay fp32 until the
bf16 p-tile write. Host pre-casts all inputs to bf16 so no device-side casts.
"""
import numpy as np
import ml_dtypes

import concourse.bacc as bacc
import concourse.mybir as mybir
import concourse.tile as tile
from concourse.bass_utils import run_bass_kernel_spmd
from concourse.masks import make_identity

F32 = mybir.dt.float32
BF16 = mybir.dt.bfloat16
NPBF16 = ml_dtypes.bfloat16

B, T, D = 4, 2048, 1024
NH, NKV, HD = 8, 1, 128
SOFTCAP = 30.0
NHL = 4            # heads per core
CH = 512           # q-chunk size
NCH = T // CH      # 4 chunks
NKT = D // 128     # 8 k-tiles over D
NTT = T // 128     # 16 token tiles


def _build_nc():
    nc = bacc.Bacc()

    xT = nc.dram_tensor("xT", [D, T], BF16, kind="ExternalInput")
    wqT = nc.dram_tensor("wqT", [D, NHL * HD], BF16, kind="ExternalInput")
    wkT = nc.dram_tensor("wkT", [D, HD], BF16, kind="ExternalInput")
    wvT = nc.dram_tensor("wvT", [D, HD], BF16, kind="ExternalInput")
    woT = nc.dram_tensor("woT", [NHL * HD, D], BF16, kind="ExternalInput")
    cc = nc.dram_tensor("cc", [HD, T], F32, kind="ExternalInput")
    ss = nc.dram_tensor("ss", [HD, T], F32, kind="ExternalInput")
    masks = nc.dram_tensor("masks", [128, 4 * CH], BF16, kind="ExternalInput")
    onesv = nc.dram_tensor("onesv", [128, 1], BF16, kind="ExternalInput")
    out = nc.dram_tensor("out", [T, D], F32, kind="ExternalOutput")

    # DRAM views tiled by 128 along the leading dim.
    xT_t = xT.rearrange("(kt p) t -> p kt t", p=128)      # [128, 8, 2048]
    wqT_t = wqT.rearrange("(kt p) c -> p kt c", p=128)    # [128, 8, 512]
    wkT_t = wkT.rearrange("(kt p) c -> p kt c", p=128)    # [128, 8, 128]
    wvT_t = wvT.rearrange("(kt p) c -> p kt c", p=128)    # [128, 8, 128]
    woT_t = woT.rearrange("(h p) c -> p h c", p=128)      # [128, 4, 1024]

    with tile.TileContext(nc) as tc:
        with (
            tc.tile_pool(name="persist", bufs=1) as persist,
            tc.tile_pool(name="wpool", bufs=1) as wpool,
            tc.tile_pool(name="xc_pool", bufs=2) as xc_pool,
            tc.tile_pool(name="qt_pool", bufs=2) as qt_pool,
            tc.tile_pool(name="rope_pool", bufs=3) as rope_pool,
            tc.tile_pool(name="t_pool", bufs=2) as t_pool,
            tc.tile_pool(name="p_pool", bufs=3) as p_pool,
            tc.tile_pool(name="avn_pool", bufs=6) as avn_pool,
            tc.tile_pool(name="norm_pool", bufs=2) as norm_pool,
            tc.tile_pool(name="osb_pool", bufs=3) as osb_pool,
            tc.tile_pool(name="s_pool", bufs=3, space="PSUM") as s_pool,
            tc.tile_pool(name="acc_pool", bufs=2, space="PSUM") as acc_pool,
            tc.tile_pool(name="d_pool", bufs=1, space="PSUM") as d_pool,
            tc.tile_pool(name="pj_pool", bufs=2, space="PSUM") as pj_pool,
        ):
            # --- constants & weights ---
            wq_sb = wpool.tile([128, NKT, NHL * HD], BF16)
            wk_sb = wpool.tile([128, NKT, HD], BF16)
            wv_sb = wpool.tile([128, NKT, HD], BF16)
            wo_sb = wpool.tile([128, NHL, D], BF16)
            cc_sb = wpool.tile([HD, T], F32)
            ss_sb = wpool.tile([HD, T], F32)
            mask_sb = wpool.tile([128, 4 * CH], BF16)
            ones_sb = wpool.tile([128, 1], BF16)
            nc.sync.dma_start(wq_sb[:], wqT_t)
            nc.sync.dma_start(wk_sb[:], wkT_t)
            nc.sync.dma_start(wv_sb[:], wvT_t)
            nc.sync.dma_start(wo_sb[:], woT_t)
            nc.sync.dma_start(cc_sb[:], cc[:])
            nc.sync.dma_start(ss_sb[:], ss[:])
            nc.sync.dma_start(mask_sb[:], masks[:])
            nc.sync.dma_start(ones_sb[:], onesv[:])

            ident = persist.tile([128, 128], F32)
            make_identity(nc, ident[:])
            negcap = persist.tile([128, 1], F32)
            nc.gpsimd.memset(negcap[:], -SOFTCAP)

            # K^T [hd, T] and V [tok, hd] persist for the whole sequence.
            kT_sb = persist.tile([HD, T], BF16)
            v_sb = persist.tile([128, NTT, HD], BF16)

            def rope_to(dst_ap, src_ps, c):
                """dst = rope(src) for a [128, CH] chunk at token offset c*CH."""
                csl = slice(c * CH, (c + 1) * CH)
                swp = rope_pool.tile([128, CH], F32, tag="swp", name="swp")
                nc.vector.tensor_copy(swp[0:64, :], src_ps[64:128, :])
                nc.vector.tensor_copy(swp[64:128, :], src_ps[0:64, :])
                m1 = rope_pool.tile([128, CH], F32, tag="m1", name="m1")
                nc.vector.tensor_mul(m1[:], src_ps[:], cc_sb[:, csl])
                m2 = rope_pool.tile([128, CH], F32, tag="m2", name="m2")
                nc.vector.tensor_mul(m2[:], swp[:], ss_sb[:, csl])
                nc.vector.tensor_add(dst_ap, m1[:], m2[:])

            for c in range(NCH):
                csl = slice(c * CH, (c + 1) * CH)

                # ---- projections for this token chunk ----
                xc = xc_pool.tile([128, NKT, CH], BF16, tag="xc", name="xc")
                nc.sync.dma_start(xc[:], xT_t[:, :, csl])

                # Q^T per head (rope'd); only this chunk's queries are needed.
                qt = qt_pool.tile([HD, NHL, CH], BF16, tag="qt", name="qt")
                for h in range(NHL):
                    q_ps = pj_pool.tile([HD, CH], F32, tag="pj", name="q_ps")
                    for kt in range(NKT):
                        nc.tensor.matmul(
                            q_ps[:], wq_sb[:, kt, h * HD:(h + 1) * HD],
                            xc[:, kt, :], start=(kt == 0), stop=(kt == NKT - 1))
                    rope_to(qt[:, h, :], q_ps, c)

                # K^T chunk (rope'd)
                k_ps = pj_pool.tile([HD, CH], F32, tag="pj", name="k_ps")
                for kt in range(NKT):
                    nc.tensor.matmul(k_ps[:], wk_sb[:, kt, :], xc[:, kt, :],
                                     start=(kt == 0), stop=(kt == NKT - 1))
                rope_to(kT_sb[:, csl], k_ps, c)

                # V chunk: compute V^T, then PE-transpose into [tok, hd] tiles.
                v_ps = pj_pool.tile([HD, CH], F32, tag="pj", name="v_ps")
                for kt in range(NKT):
                    nc.tensor.matmul(v_ps[:], wv_sb[:, kt, :], xc[:, kt, :],
                                     start=(kt == 0), stop=(kt == NKT - 1))
                vt_sb = rope_pool.tile([HD, CH], F32, tag="vt", name="vt_sb")
                nc.vector.tensor_copy(vt_sb[:], v_ps[:])
                for tt in range(CH // 128):
                    tp_ps = pj_pool.tile([128, 128], F32, tag="pj", name="tp_ps")
                    nc.tensor.transpose(
                        tp_ps[:], vt_sb[:, tt * 128:(tt + 1) * 128], ident[:])
                    nc.vector.tensor_copy(v_sb[:, c * 4 + tt, :], tp_ps[:])

                # ---- attention for this q-chunk ----
                avn_tiles = []
                for h in range(NHL):
                    av_ps = acc_pool.tile([HD, CH], F32, tag="av", name="av_ps")
                    d_ps = d_pool.tile([1, CH], F32, tag="d", name="d_ps")
                    nkb = 4 * (c + 1)
                    for g in range(c + 1):      # groups of 4 k-blocks
                        t4 = t_pool.tile([128, 4 * CH], F32, tag="t4", name="t4")
                        for j in range(4):
                            kb = 4 * g + j
                            sp = s_pool.tile([128, CH], F32, tag="s1", name="sp")
                            nc.tensor.matmul(
                                sp[:], kT_sb[:, kb * 128:(kb + 1) * 128],
                                qt[:, h, :], start=True, stop=True)
                            nc.scalar.activation(
                                t4[:, j * CH:(j + 1) * CH], sp[:],
                                mybir.ActivationFunctionType.Tanh)
                        p4 = p_pool.tile([128, 4 * CH], BF16, tag="p4", name="p4")
                        nc.scalar.activation(
                            p4[:], t4[:], mybir.ActivationFunctionType.Exp,
                            scale=SOFTCAP, bias=negcap[:])
                        if g == c:  # diagonal band: exact multiplicative mask
                            p4m = p_pool.tile([128, 4 * CH], BF16, tag="p4m",
                                              name="p4m")
                            nc.vector.tensor_mul(p4m[:], p4[:], mask_sb[:])
                            p4 = p4m
                        for j in range(4):
                            kb = 4 * g + j
                            pj_ap = p4[:, j * CH:(j + 1) * CH]
                            nc.tensor.matmul(av_ps[:], v_sb[:, kb, :], pj_ap,
                                             start=(kb == 0),
                                             stop=(kb == nkb - 1))
                            nc.tensor.matmul(d_ps[:], ones_sb[:], pj_ap,
                                             start=(kb == 0),
                                             stop=(kb == nkb - 1))
                    dinv = norm_pool.tile([128, CH], F32, tag="dinv", name="dinv")
                    nc.vector.reciprocal_approx_fast(dinv[0:1, :], d_ps[:])
                    dbc = norm_pool.tile([128, CH], F32, tag="dbc", name="dbc")
                    nc.gpsimd.partition_broadcast(dbc[:], dinv[:])
                    avn = avn_pool.tile([HD, CH], BF16, tag="avn", name="avn")
                    nc.vector.tensor_mul(avn[:], av_ps[:], dbc[:])
                    avn_tiles.append(avn)

                # ---- Wo partials for this chunk: out[tok, :] = sum_h ----
                for tt in range(CH // 128):
                    for dc in range(2):
                        o_ps = pj_pool.tile([128, CH], F32, tag="pj", name="o_ps")
                        for h in range(NHL):
                            nc.tensor.matmul(
                                o_ps[:],
                                avn_tiles[h][:, tt * 128:(tt + 1) * 128],
                                wo_sb[:, h, dc * CH:(dc + 1) * CH],
                                start=(h == 0), stop=(h == NHL - 1))
                        o_sb = osb_pool.tile([128, CH], F32, tag="osb", name="o_sb")
                        nc.vector.tensor_copy(o_sb[:], o_ps[:])
                        nc.sync.dma_start(
                            out[c * CH + tt * 128: c * CH + (tt + 1) * 128,
                                dc * CH:(dc + 1) * CH], o_sb[:])

    nc.compile()
    return nc


_CACHED_NC = None


def _get_nc():
    global _CACHED_NC
    if _CACHED_NC is None:
        _CACHED_NC = _build_nc()
    return _CACHED_NC


def _host_inputs(x, Wq, Wk, Wv, Wo, qk_gain, cos, sin):
    """Build the 8 per-core input maps (bf16 matmul operands, fp32 rope tables)."""
    x = np.asarray(x, np.float32)
    Wq = np.asarray(Wq, np.float32)
    Wk = np.asarray(Wk, np.float32)
    Wv = np.asarray(Wv, np.float32)
    Wo = np.asarray(Wo, np.float32)
    qk_gain = np.asarray(qk_gain, np.float32)
    cos = np.asarray(cos, np.float32)
    sin = np.asarray(sin, np.float32)

    scale = 1.0 / (np.sqrt(HD) * SOFTCAP)
    # Fold per-head gain and softcap scale into Wq rows.
    Wq_s = Wq * (qk_gain[:, None].repeat(HD, 1).reshape(NH * HD, 1) * scale)

    wkT = np.ascontiguousarray(Wk.T.astype(NPBF16))
    wvT = np.ascontiguousarray(Wv.T.astype(NPBF16))
    cosT = cos.T  # [64, T]
    sinT = sin.T
    cc = np.ascontiguousarray(np.concatenate([cosT, cosT], 0))          # [128, T]
    ss = np.ascontiguousarray(np.concatenate([-sinT, sinT], 0))         # [128, T]

    # Diagonal-band causal masks: for residue r, mask[kk, qq] = qq >= 128*r + kk
    qq = np.arange(CH)
    kk = np.arange(128)
    masks = np.empty((128, 4 * CH), np.float32)
    for r in range(4):
        masks[:, r * CH:(r + 1) * CH] = (qq[None, :] >= (128 * r + kk)[:, None])
    masks = masks.astype(NPBF16)
    onesv = np.ones((128, 1), NPBF16)

    xTs = [np.ascontiguousarray(x[b].T.astype(NPBF16)) for b in range(B)]
    in_maps = []
    for core in range(8):
        b, hh = divmod(core, 2)
        h0 = hh * NHL
        wqT = np.ascontiguousarray(Wq_s[h0 * HD:(h0 + NHL) * HD, :].T.astype(NPBF16))
        woT = np.ascontiguousarray(Wo[:, h0 * HD:(h0 + NHL) * HD].T.astype(NPBF16))
        in_maps.append({
            "xT": xTs[b], "wqT": wqT, "wkT": wkT, "wvT": wvT, "woT": woT,
            "cc": cc, "ss": ss, "masks": masks, "onesv": onesv,
        })
    return in_maps


def kernel(x, Wq, Wk, Wv, Wo, qk_gain, cos, sin, _trace=False):
    in_maps = _host_inputs(x, Wq, Wk, Wv, Wo, qk_gain, cos, sin)
    nc = _get_nc()
    res = run_bass_kernel_spmd(nc, in_maps, core_ids=list(range(8)),
                               trace=_trace)
    out = np.empty((B, T, D), np.float32)
    for b in range(B):
        out[b] = res.results[2 * b]["out"] + res.results[2 * b + 1]["out"]
    if _trace:
        kernel.last_exec_time_ns = res.exec_time_ns
        kernel.last_results = res
    return out


# revision 8
# speedup vs baseline: 1.4750x; 1.2569x over previous
"""Causal GQA attention (nkv=1) with RoPE + logit softcap, sharded over 8 trn2 cores.

Sharding: core = 2*b + hh  (b = batch 0..3, hh = head-half 0..1).
Each core computes, for its batch b and its 4 query heads:
  q = rope(x @ Wq_h'.T)          (gain/(sqrt(hd)*softcap) folded into Wq on host)
  k = rope(x @ Wk.T), v = x @ Wv.T   (single kv head, shared across its 4 q heads)
  pT[k,q] = exp(softcap*tanh(qT.k) - softcap) * causal_mask   (max-free softmax:
            softcap bounds logits to +-30 so exp never overflows)
  outT_h = (v.T @ pT) / sum_k pT    accumulated in PSUM; denominator via ones-matmul
  partial_out[tok, :] = sum_h outT_h.T @ Wo[:, head cols].T
Host sums the two half-head partials per batch and stacks batches.

v2: all matmuls in bf16 (1 cyc/row, keeps the PE HAM-warm; fp32r lowers to
fp32_mode=HIGH which runs ~2x slower and HAM-oscillates). Scores accumulate in
fp32 PSUM; softmax numerics (tanh/exp on ACT, fp32 in) stay fp32 until the
bf16 p-tile write. Host pre-casts all inputs to bf16 so no device-side casts.
"""
import numpy as np
import ml_dtypes

import concourse.bacc as bacc
import concourse.mybir as mybir
import concourse.tile as tile
from concourse.bass_utils import run_bass_kernel_spmd
from concourse.masks import make_identity

F32 = mybir.dt.float32
BF16 = mybir.dt.bfloat16
NPBF16 = ml_dtypes.bfloat16

B, T, D = 4, 2048, 1024
NH, NKV, HD = 8, 1, 128
SOFTCAP = 30.0
NHL = 4            # heads per core
CH = 512           # q-chunk size
NCH = T // CH      # 4 chunks
NKT = D // 128     # 8 k-tiles over D
NTT = T // 128     # 16 token tiles


def _build_nc():
    nc = bacc.Bacc()

    xT = nc.dram_tensor("xT", [D, T], BF16, kind="ExternalInput")
    wqT = nc.dram_tensor("wqT", [D, NHL * HD], BF16, kind="ExternalInput")
    wkT = nc.dram_tensor("wkT", [D, HD], BF16, kind="ExternalInput")
    wvT = nc.dram_tensor("wvT", [D, HD], BF16, kind="ExternalInput")
    woT = nc.dram_tensor("woT", [NHL * HD, D], BF16, kind="ExternalInput")
    cc = nc.dram_tensor("cc", [HD, T], F32, kind="ExternalInput")
    ss = nc.dram_tensor("ss", [HD, T], F32, kind="ExternalInput")
    masks = nc.dram_tensor("masks", [128, 4 * CH], BF16, kind="ExternalInput")
    onesv = nc.dram_tensor("onesv", [128, 1], BF16, kind="ExternalInput")
    out = nc.dram_tensor("out", [T, D], F32, kind="ExternalOutput")

    # DRAM views tiled by 128 along the leading dim.
    xT_t = xT.rearrange("(kt p) t -> p kt t", p=128)      # [128, 8, 2048]
    wqT_t = wqT.rearrange("(kt p) c -> p kt c", p=128)    # [128, 8, 512]
    wkT_t = wkT.rearrange("(kt p) c -> p kt c", p=128)    # [128, 8, 128]
    wvT_t = wvT.rearrange("(kt p) c -> p kt c", p=128)    # [128, 8, 128]
    woT_t = woT.rearrange("(h p) c -> p h c", p=128)      # [128, 4, 1024]

    with tile.TileContext(nc) as tc:
        with (
            tc.tile_pool(name="persist", bufs=1) as persist,
            tc.tile_pool(name="wpool", bufs=1) as wpool,
            tc.tile_pool(name="xc_pool", bufs=2) as xc_pool,
            tc.tile_pool(name="qt_pool", bufs=2) as qt_pool,
            tc.tile_pool(name="rope_pool", bufs=3) as rope_pool,
            tc.tile_pool(name="t_pool", bufs=2) as t_pool,
            tc.tile_pool(name="p_pool", bufs=3) as p_pool,
            tc.tile_pool(name="avn_pool", bufs=6) as avn_pool,
            tc.tile_pool(name="norm_pool", bufs=2) as norm_pool,
            tc.tile_pool(name="osb_pool", bufs=3) as osb_pool,
            tc.tile_pool(name="s_pool", bufs=2, space="PSUM") as s_pool,
            tc.tile_pool(name="acc_pool", bufs=1, space="PSUM") as acc_pool,
            tc.tile_pool(name="d_pool", bufs=1, space="PSUM") as d_pool,
            tc.tile_pool(name="pj_pool", bufs=2, space="PSUM") as pj_pool,
        ):
            # --- constants & weights ---
            wq_sb = wpool.tile([128, NKT, NHL * HD], BF16)
            wk_sb = wpool.tile([128, NKT, HD], BF16)
            wv_sb = wpool.tile([128, NKT, HD], BF16)
            wo_sb = wpool.tile([128, NHL, D], BF16)
            cc_sb = wpool.tile([HD, T], F32)
            ss_sb = wpool.tile([HD, T], F32)
            mask_sb = wpool.tile([128, 4 * CH], BF16)
            ones_sb = wpool.tile([128, 1], BF16)
            nc.sync.dma_start(wq_sb[:], wqT_t)
            nc.sync.dma_start(wk_sb[:], wkT_t)
            nc.sync.dma_start(wv_sb[:], wvT_t)
            nc.sync.dma_start(wo_sb[:], woT_t)
            nc.sync.dma_start(cc_sb[:], cc[:])
            nc.sync.dma_start(ss_sb[:], ss[:])
            nc.sync.dma_start(mask_sb[:], masks[:])
            nc.sync.dma_start(ones_sb[:], onesv[:])

            ident = persist.tile([128, 128], F32)
            make_identity(nc, ident[:])
            negcap = persist.tile([128, 1], F32)
            nc.gpsimd.memset(negcap[:], -SOFTCAP)

            # K^T [hd, T] and V [tok, hd] persist for the whole sequence.
            kT_sb = persist.tile([HD, T], BF16)
            v_sb = persist.tile([128, NTT, HD], BF16)

            def rope_to(dst_ap, src_ps, c):
                """dst = rope(src) for a [128, CH] chunk at token offset c*CH."""
                csl = slice(c * CH, (c + 1) * CH)
                swp = rope_pool.tile([128, CH], F32, tag="swp", name="swp")
                nc.vector.tensor_copy(swp[0:64, :], src_ps[64:128, :])
                nc.vector.tensor_copy(swp[64:128, :], src_ps[0:64, :])
                m1 = rope_pool.tile([128, CH], F32, tag="m1", name="m1")
                nc.vector.tensor_mul(m1[:], src_ps[:], cc_sb[:, csl])
                m2 = rope_pool.tile([128, CH], F32, tag="m2", name="m2")
                nc.vector.tensor_mul(m2[:], swp[:], ss_sb[:, csl])
                nc.vector.tensor_add(dst_ap, m1[:], m2[:])

            def emit_proj(c):
                csl = slice(c * CH, (c + 1) * CH)
                xc = xc_pool.tile([128, NKT, CH], BF16, tag="xc", name="xc")
                nc.sync.dma_start(xc[:], xT_t[:, :, csl])

                # Q^T per head (rope'd); only this chunk's queries are needed.
                qt = qt_pool.tile([HD, NHL, CH], BF16, tag="qt", name="qt")
                for h in range(NHL):
                    q_ps = pj_pool.tile([HD, CH], F32, tag="pj", name="q_ps")
                    for kt in range(NKT):
                        nc.tensor.matmul(
                            q_ps[:], wq_sb[:, kt, h * HD:(h + 1) * HD],
                            xc[:, kt, :], start=(kt == 0), stop=(kt == NKT - 1))
                    rope_to(qt[:, h, :], q_ps, c)

                # K^T chunk (rope'd)
                k_ps = pj_pool.tile([HD, CH], F32, tag="pj", name="k_ps")
                for kt in range(NKT):
                    nc.tensor.matmul(k_ps[:], wk_sb[:, kt, :], xc[:, kt, :],
                                     start=(kt == 0), stop=(kt == NKT - 1))
                rope_to(kT_sb[:, csl], k_ps, c)

                # V chunk: compute V^T, then PE-transpose into [tok, hd] tiles.
                v_ps = pj_pool.tile([HD, CH], F32, tag="pj", name="v_ps")
                for kt in range(NKT):
                    nc.tensor.matmul(v_ps[:], wv_sb[:, kt, :], xc[:, kt, :],
                                     start=(kt == 0), stop=(kt == NKT - 1))
                vt_sb = rope_pool.tile([HD, CH], F32, tag="vt", name="vt_sb")
                nc.vector.tensor_copy(vt_sb[:], v_ps[:])
                for tt in range(CH // 128):
                    tp_ps = pj_pool.tile([128, 128], F32, tag="pj", name="tp_ps")
                    nc.tensor.transpose(
                        tp_ps[:], vt_sb[:, tt * 128:(tt + 1) * 128], ident[:])
                    nc.vector.tensor_copy(v_sb[:, c * 4 + tt, :], tp_ps[:])
                return qt

            def emit_attn(c, qt):
                avn_tiles = []
                for h in range(NHL):
                    av_ps = acc_pool.tile([HD, CH], F32, tag="av", name="av_ps")
                    d_ps = d_pool.tile([1, CH], F32, tag="d", name="d_ps")
                    nkb = 4 * (c + 1)
                    for g in range(c + 1):      # groups of 4 k-blocks
                        t4 = t_pool.tile([128, 4 * CH], F32, tag="t4", name="t4")
                        for half in range(2):
                            sp = s_pool.tile([128, 2 * CH], F32, tag="s2",
                                             name="sp")
                            for j in range(2):
                                kb = 4 * g + 2 * half + j
                                nc.tensor.matmul(
                                    sp[:, j * CH:(j + 1) * CH],
                                    kT_sb[:, kb * 128:(kb + 1) * 128],
                                    qt[:, h, :], start=True, stop=True)
                            nc.scalar.activation(
                                t4[:, half * 2 * CH:(half + 1) * 2 * CH],
                                sp[:], mybir.ActivationFunctionType.Tanh)
                        p4 = p_pool.tile([128, 4 * CH], BF16, tag="p4", name="p4")
                        nc.scalar.activation(
                            p4[:], t4[:], mybir.ActivationFunctionType.Exp,
                            scale=SOFTCAP, bias=negcap[:])
                        if g == c:  # diagonal band: exact multiplicative mask
                            p4m = p_pool.tile([128, 4 * CH], BF16, tag="p4m",
                                              name="p4m")
                            nc.vector.tensor_mul(p4m[:], p4[:], mask_sb[:])
                            p4 = p4m
                        for j in range(4):
                            kb = 4 * g + j
                            pj_ap = p4[:, j * CH:(j + 1) * CH]
                            nc.tensor.matmul(av_ps[:], v_sb[:, kb, :], pj_ap,
                                             start=(kb == 0),
                                             stop=(kb == nkb - 1))
                            nc.tensor.matmul(d_ps[:], ones_sb[:], pj_ap,
                                             start=(kb == 0),
                                             stop=(kb == nkb - 1))
                    dinv = norm_pool.tile([128, CH], F32, tag="dinv", name="dinv")
                    nc.vector.reciprocal_approx_fast(dinv[0:1, :], d_ps[:])
                    dbc = norm_pool.tile([128, CH], F32, tag="dbc", name="dbc")
                    nc.gpsimd.partition_broadcast(dbc[:], dinv[:])
                    avn = avn_pool.tile([HD, CH], BF16, tag="avn", name="avn")
                    nc.vector.tensor_mul(avn[:], av_ps[:], dbc[:])
                    avn_tiles.append(avn)
                return avn_tiles

            def emit_wo(c, avn_tiles):
                # ---- Wo partials for this chunk: out[tok, :] = sum_h ----
                for tt in range(CH // 128):
                    for dc in range(2):
                        o_ps = pj_pool.tile([128, CH], F32, tag="pj", name="o_ps")
                        for h in range(NHL):
                            nc.tensor.matmul(
                                o_ps[:],
                                avn_tiles[h][:, tt * 128:(tt + 1) * 128],
                                wo_sb[:, h, dc * CH:(dc + 1) * CH],
                                start=(h == 0), stop=(h == NHL - 1))
                        o_sb = osb_pool.tile([128, CH], F32, tag="osb", name="o_sb")
                        nc.vector.tensor_copy(o_sb[:], o_ps[:])
                        nc.sync.dma_start(
                            out[c * CH + tt * 128: c * CH + (tt + 1) * 128,
                                dc * CH:(dc + 1) * CH], o_sb[:])

            qt = emit_proj(0)
            for c in range(NCH):
                avn_tiles = emit_attn(c, qt)
                if c + 1 < NCH:
                    qt = emit_proj(c + 1)
                emit_wo(c, avn_tiles)

    nc.compile()
    return nc


_CACHED_NC = None


def _get_nc():
    global _CACHED_NC
    if _CACHED_NC is None:
        _CACHED_NC = _build_nc()
    return _CACHED_NC


def _host_inputs(x, Wq, Wk, Wv, Wo, qk_gain, cos, sin):
    """Build the 8 per-core input maps (bf16 matmul operands, fp32 rope tables)."""
    x = np.asarray(x, np.float32)
    Wq = np.asarray(Wq, np.float32)
    Wk = np.asarray(Wk, np.float32)
    Wv = np.asarray(Wv, np.float32)
    Wo = np.asarray(Wo, np.float32)
    qk_gain = np.asarray(qk_gain, np.float32)
    cos = np.asarray(cos, np.float32)
    sin = np.asarray(sin, np.float32)

    scale = 1.0 / (np.sqrt(HD) * SOFTCAP)
    # Fold per-head gain and softcap scale into Wq rows.
    Wq_s = Wq * (qk_gain[:, None].repeat(HD, 1).reshape(NH * HD, 1) * scale)

    wkT = np.ascontiguousarray(Wk.T.astype(NPBF16))
    wvT = np.ascontiguousarray(Wv.T.astype(NPBF16))
    cosT = cos.T  # [64, T]
    sinT = sin.T
    cc = np.ascontiguousarray(np.concatenate([cosT, cosT], 0))          # [128, T]
    ss = np.ascontiguousarray(np.concatenate([-sinT, sinT], 0))         # [128, T]

    # Diagonal-band causal masks: for residue r, mask[kk, qq] = qq >= 128*r + kk
    qq = np.arange(CH)
    kk = np.arange(128)
    masks = np.empty((128, 4 * CH), np.float32)
    for r in range(4):
        masks[:, r * CH:(r + 1) * CH] = (qq[None, :] >= (128 * r + kk)[:, None])
    masks = masks.astype(NPBF16)
    onesv = np.ones((128, 1), NPBF16)

    xTs = [np.ascontiguousarray(x[b].T.astype(NPBF16)) for b in range(B)]
    in_maps = []
    for core in range(8):
        b, hh = divmod(core, 2)
        h0 = hh * NHL
        wqT = np.ascontiguousarray(Wq_s[h0 * HD:(h0 + NHL) * HD, :].T.astype(NPBF16))
        woT = np.ascontiguousarray(Wo[:, h0 * HD:(h0 + NHL) * HD].T.astype(NPBF16))
        in_maps.append({
            "xT": xTs[b], "wqT": wqT, "wkT": wkT, "wvT": wvT, "woT": woT,
            "cc": cc, "ss": ss, "masks": masks, "onesv": onesv,
        })
    return in_maps


def kernel(x, Wq, Wk, Wv, Wo, qk_gain, cos, sin, _trace=False):
    in_maps = _host_inputs(x, Wq, Wk, Wv, Wo, qk_gain, cos, sin)
    nc = _get_nc()
    res = run_bass_kernel_spmd(nc, in_maps, core_ids=list(range(8)),
                               trace=_trace)
    out = np.empty((B, T, D), np.float32)
    for b in range(B):
        out[b] = res.results[2 * b]["out"] + res.results[2 * b + 1]["out"]
    if _trace:
        kernel.last_exec_time_ns = res.exec_time_ns
        kernel.last_results = res
    return out
